# revision 15
# baseline (speedup 1.0000x reference)
"""Multi-head attention Trainium2 kernel (B=8, N=1024, C=768, H=12, D=64).

Sharding: data-parallel over batch — one batch element per NeuronCore (8 cores).
Returns (out [8,1024,768], attn [8,12,1024,1024]) matching the reference.

Layout strategy (per core, batch b):
  - host passes x[b].T and W.T (bf16) so every matmul contracts over partitions
  - qT,kT [C,N] and v [N,C] via projections; per head pair the S and S^T
    logits are computed packed two-heads-per-PE-array (row groups 0-1 / 2-3)
  - attn = exp(S*scale) * recip(rowsum) with rowsums from ACT accum_out
  - attn@v uses exp(S^T) tiles; an all-ones lhsT col-packed matmul makes the
    PE broadcast the softmax sums into the same psum partitions for a
    partition-aligned normalize
  - out = attn_out @ Wp.T + bias via K=1 ones-row matmul
"""

import numpy as np

B, N, C, H, D = 8, 1024, 768, 12, 64
SCALE = D ** -0.5  # 0.125
NT = N // 128  # 8 n-tiles
CT = C // 128  # 6 c-tiles (= head pairs)

_CACHE = {}


def _build_program(bench_loop=None):
    from contextlib import ExitStack

    import concourse.bacc as bacc
    import concourse.mybir as mybir
    import concourse.tile as tile

    fp32 = mybir.dt.float32
    bf16 = mybir.dt.bfloat16
    Exp = mybir.ActivationFunctionType.Exp

    nc = bacc.Bacc()
    xt = nc.declare_dram_parameter("xt", [C, N], bf16, isOutput=False)  # x[b].T
    wq = nc.declare_dram_parameter("wq", [C, C], bf16, isOutput=False)  # Wq.T
    wk = nc.declare_dram_parameter("wk", [C, C], bf16, isOutput=False)  # Wk.T
    wv = nc.declare_dram_parameter("wv", [C, C], bf16, isOutput=False)  # Wv.T
    wp = nc.declare_dram_parameter("wp", [C, C], bf16, isOutput=False)  # Wp.T
    bp = nc.declare_dram_parameter("bp", [1, C], bf16, isOutput=False)
    out = nc.declare_dram_parameter("out", [N, C], fp32, isOutput=True)
    attn = nc.declare_dram_parameter("attn", [H, N, N], fp32, isOutput=True)

    with ExitStack() as ctx:
        tc = ctx.enter_context(tile.TileContext(nc))
        if bench_loop is not None:
            ctx.enter_context(
                tc.For_i(0, bench_loop, 1,
                         hint_engines=(mybir.EngineType.PE,
                                       mybir.EngineType.Activation,
                                       mybir.EngineType.DVE))
            )

        qk_pool = ctx.enter_context(tc.tile_pool(name="qk", bufs=6))
        v_pool = ctx.enter_context(tc.tile_pool(name="vp", bufs=8))
        aot_pool = ctx.enter_context(tc.tile_pool(name="aot", bufs=6))
        wpb_pool = ctx.enter_context(tc.tile_pool(name="wpb", bufs=6))
        const_pool = ctx.enter_context(tc.tile_pool(name="cst", bufs=1))
        xw_pool = ctx.enter_context(tc.tile_pool(name="xw", bufs=6))
        ph2 = ctx.enter_context(tc.tile_pool(name="ph2", bufs=1))
        sml = ctx.enter_context(tc.tile_pool(name="sml", bufs=8))
        rrp = ctx.enter_context(tc.tile_pool(name="rrp", bufs=4))

        # PSUM: "s" 3x[128,1024] = 6 banks; "av" 2x[128,512] = 2 banks
        ps_pool = ctx.enter_context(tc.tile_pool(name="ps", bufs=3, space="PSUM"))
        av_pool = ctx.enter_context(tc.tile_pool(name="avp", bufs=2, space="PSUM"))

        qT = [qk_pool.tile([128, N], bf16, tag="qT", name=f"qT{i}") for i in range(CT)]
        kT = [qk_pool.tile([128, N], bf16, tag="kT", name=f"kT{i}") for i in range(CT)]
        v_sb = [v_pool.tile([128, C], bf16, tag="v", name=f"v{i}") for i in range(NT)]
        aot = [aot_pool.tile([128, N], bf16, tag="aot", name=f"aot{i}") for i in range(CT)]
        wp_sb = [wpb_pool.tile([128, C], bf16, tag="wp", name=f"wp{i}") for i in range(CT)]
        bp_sb = wpb_pool.tile([1, C], bf16, tag="bp", bufs=1)
        ones64 = const_pool.tile([128, 64], bf16, tag="o64", bufs=1)
        onesrow = const_pool.tile([1, N], bf16, tag="orow", bufs=1)
        xt_sb = [xw_pool.tile([128, N], bf16, tag="xt", name=f"xts{i}") for i in range(CT)]
        wq_sb = [xw_pool.tile([128, C], bf16, tag="wq", name=f"wqs{i}") for i in range(CT)]
        wk_sb = [xw_pool.tile([128, C], bf16, tag="wk", name=f"wks{i}") for i in range(CT)]
        wv_sb = [xw_pool.tile([128, C], bf16, tag="wv", name=f"wvs{i}") for i in range(CT)]

        nc.vector.memset(ones64, 1.0)
        nc.vector.memset(onesrow, 1.0)
        for t in range(CT):
            nc.sync.dma_start(out=xt_sb[t], in_=xt[t * 128 : (t + 1) * 128, :])
            nc.sync.dma_start(out=wq_sb[t], in_=wq[t * 128 : (t + 1) * 128, :])
            nc.sync.dma_start(out=wk_sb[t], in_=wk[t * 128 : (t + 1) * 128, :])
        nc.sync.dma_start(out=bp_sb, in_=bp[:, :])
        for t in range(CT):
            nc.sync.dma_start(out=wv_sb[t], in_=wv[t * 128 : (t + 1) * 128, :])
            nc.sync.dma_start(out=wp_sb[t], in_=wp[t * 128 : (t + 1) * 128, :])

        def qk_group(ct, g):
            # g in 0..3: (c2, q-or-k) = (g//2, g%2)
            csl = slice(ct * 128, (ct + 1) * 128)
            c2, which = g // 2, g % 2
            nsl = slice(c2 * 512, (c2 + 1) * 512)
            w_sb, dst = (wq_sb, qT) if which == 0 else (wk_sb, kT)
            p = ps_pool.tile([128, N], fp32, tag="s", name=f"p{which}{ct}{c2}")
            for kt_ in range(CT):
                nc.tensor.matmul(
                    p[:, 0:512], w_sb[kt_][:, csl], xt_sb[kt_][:, nsl],
                    start=(kt_ == 0), stop=(kt_ == CT - 1),
                )
            nc.vector.tensor_copy(dst[ct][:, nsl], p[:, 0:512])

        def qk_proj(ct):
            for g in range(4):
                qk_group(ct, g)

        def v_proj_mt(mt):
            pv = ps_pool.tile([128, C], fp32, tag="s", name=f"pv{mt}")
            msl = slice(mt * 128, (mt + 1) * 128)
            for kt_ in range(CT):
                nc.tensor.matmul(
                    pv[:, 0:512], xt_sb[kt_][:, msl], wv_sb[kt_][:, 0:512],
                    start=(kt_ == 0), stop=(kt_ == CT - 1),
                )
                nc.tensor.matmul(
                    pv[:, 512:768], xt_sb[kt_][:, msl], wv_sb[kt_][:, 512:768],
                    start=(kt_ == 0), stop=(kt_ == CT - 1),
                )
            nc.vector.tensor_copy(v_sb[mt], pv)

        def s_side(hp, extra=None):
            qt_t, kt_t = qT[hp], kT[hp]
            for nt in range(NT):
                if extra is not None and nt % 2 == 1:
                    extra(nt // 2)
                nsl = slice(nt * 128, (nt + 1) * 128)
                pse = ps_pool.tile([128, N], fp32, tag="s", name=f"se{hp}{nt}")
                pso = ps_pool.tile([128, N], fp32, tag="s", name=f"so{hp}{nt}")
                for c2 in range(2):
                    sl = slice(c2 * 512, (c2 + 1) * 512)
                    nc.tensor.matmul(
                        pse[:, sl], qt_t[0:64, nsl], kt_t[0:64, sl],
                        start=True, stop=True, tile_position=(0, 0),
                    )
                    nc.tensor.matmul(
                        pso[:, sl], qt_t[64:128, nsl], kt_t[64:128, sl],
                        start=True, stop=True, tile_position=(64, 0),
                    )
                for h2, pst in ((0, pse), (1, pso)):
                    h = 2 * hp + h2
                    ex = ph2.tile([128, N], fp32, tag="ex", bufs=6, name=f"ex{h}{nt}")
                    sm = sml.tile([128, 1], fp32, tag="sm", name=f"sm{h}{nt}")
                    nc.scalar.activation(
                        out=ex, in_=pst, func=Exp, scale=SCALE, accum_out=sm
                    )
                    rs = sml.tile([128, 1], fp32, tag="rs", name=f"rs{h}{nt}")
                    nc.vector.reciprocal_approx_fast(rs, sm)
                    nc.vector.tensor_scalar_mul(ex, ex, rs)
                    nc.sync.dma_start(out=attn[h, nsl, :], in_=ex)

        def st_mms(hp, mt, ests):
            qt_t, kt_t = qT[hp], kT[hp]
            msl = slice(mt * 128, (mt + 1) * 128)
            pse = ps_pool.tile([128, N], fp32, tag="s", name=f"te{hp}{mt}")
            pso = ps_pool.tile([128, N], fp32, tag="s", name=f"to{hp}{mt}")
            for c2 in range(2):
                sl = slice(c2 * 512, (c2 + 1) * 512)
                nc.tensor.matmul(
                    pse[:, sl], kt_t[0:64, msl], qt_t[0:64, sl],
                    start=True, stop=True, tile_position=(0, 0),
                )
                nc.tensor.matmul(
                    pso[:, sl], kt_t[64:128, msl], qt_t[64:128, sl],
                    start=True, stop=True, tile_position=(64, 0),
                )
            este = ph2.tile([128, N], bf16, tag="est", bufs=22, name=f"ee{hp}{mt}")
            esto = ph2.tile([128, N], bf16, tag="est", bufs=22, name=f"eo{hp}{mt}")
            nc.scalar.activation(out=este, in_=pse, func=Exp, scale=SCALE)
            nc.scalar.activation(out=esto, in_=pso, func=Exp, scale=SCALE)
            ests.append((este, esto))

        def av_quarter(hp, ests, q, state):
            # q in 0..3: (chunk, mt-half) = (q//2, q%2)
            c2, half = q // 2, q % 2
            sl = slice(c2 * 512, (c2 + 1) * 512)
            if half == 0:
                avd = av_pool.tile([128, 512], fp32, tag="av", name=f"ad{hp}{c2}")
                avs = av_pool.tile([128, 512], fp32, tag="av", name=f"as{hp}{c2}")
                state[c2] = (avd, avs)
            avd, avs = state[c2]
            for mt in range(half * 4, half * 4 + 4):
                for h2 in range(2):
                    est_t = ests[mt][h2]
                    h = 2 * hp + h2
                    rsl = slice(h2 * 64, (h2 + 1) * 64)
                    nc.tensor.matmul(
                        avd[rsl, :],
                        v_sb[mt][:, h * 64 : (h + 1) * 64],
                        est_t[:, sl],
                        start=(mt == 0), stop=(mt == NT - 1),
                        tile_position=(0, h2 * 64),
                    )
                    nc.tensor.matmul(
                        avs[rsl, :], ones64, est_t[:, sl],
                        start=(mt == 0), stop=(mt == NT - 1),
                        tile_position=(0, h2 * 64),
                    )
            if half == 1:
                rr = rrp.tile([128, 512], fp32, tag="rr", name=f"rr{hp}{c2}")
                nc.vector.reciprocal_approx_fast(rr, avs)
                nc.vector.tensor_mul(aot[hp][:, sl], avd, rr)

        partials = []

        def proj_partial(nt):
            # pairs 0..4 contribution of the output projection (runs while
            # pair 5 is still computing); aot[5] + bias join in the tail
            nsl = slice(nt * 128, (nt + 1) * 128)
            po = ps_pool.tile([128, C], fp32, tag="s", name=f"pp{nt}")
            for c2, sl in ((0, slice(0, 512)), (1, slice(512, 768))):
                for kt_ in range(CT - 1):
                    nc.tensor.matmul(
                        po[:, sl], aot[kt_][:, nsl], wp_sb[kt_][:, sl],
                        start=(kt_ == 0), stop=(kt_ == CT - 2),
                    )
            pt = ph2.tile([128, C], bf16, tag="pt", bufs=8, name=f"pt{nt}")
            nc.vector.tensor_copy(pt, po)
            partials.append(pt)

        # software-pipelined schedule:
        #   pair t's S^T loop hosts AV of pair t-1 (V projection for pair 0);
        #   pair t's S loop hosts the QK projection of pair t+1 (or, for the
        #   last pair, the first 5/6 of the output projection).
        qk_proj(0)
        prev = None  # (hp, ests) awaiting AV
        for hp in range(CT):
            ests = []
            avstate = {}
            for mt in range(NT):
                st_mms(hp, mt, ests)
                if hp == 0:
                    v_proj_mt(mt)
                elif mt < 4:
                    av_quarter(prev[0], prev[1], mt, avstate)
            if hp + 1 < CT:
                s_loop_extra = lambda g: qk_group(hp + 1, g)
            else:
                s_loop_extra = lambda g: (proj_partial(2 * g), proj_partial(2 * g + 1))
            s_side(hp, s_loop_extra)
            prev = (hp, ests)
        avstate = {}
        for q in range(4):
            av_quarter(prev[0], prev[1], q, avstate)

        # output projection tail: last pair's contribution + bias + partial
        for nt in range(NT):
            nsl = slice(nt * 128, (nt + 1) * 128)
            po = ps_pool.tile([128, C], fp32, tag="s", name=f"po{nt}")
            for c2, sl in ((0, slice(0, 512)), (1, slice(512, 768))):
                nc.tensor.matmul(
                    po[:, sl], aot[CT - 1][:, nsl], wp_sb[CT - 1][:, sl],
                    start=True, stop=False,
                )
                nc.tensor.matmul(
                    po[:, sl], onesrow[0:1, nsl], bp_sb[0:1, sl],
                    start=False, stop=True,
                )
            ot = ph2.tile([128, C], fp32, tag="ot", bufs=3, name=f"ot{nt}")
            nc.vector.tensor_add(ot, po, partials[nt])
            nc.sync.dma_start(out=out[nsl, :], in_=ot)

    nc.compile()
    return nc


def get_program(bench_loop=None):
    key = ("nc", bench_loop)
    if key not in _CACHE:
        _CACHE[key] = _build_program(bench_loop)
    return _CACHE[key]


def make_in_maps(x, Wq, Wk, Wv, Wp, bp):
    import ml_dtypes

    bf = ml_dtypes.bfloat16
    x = np.asarray(x, dtype=np.float32)
    wqT = np.ascontiguousarray(np.asarray(Wq, np.float32).T.astype(bf))
    wkT = np.ascontiguousarray(np.asarray(Wk, np.float32).T.astype(bf))
    wvT = np.ascontiguousarray(np.asarray(Wv, np.float32).T.astype(bf))
    wpT = np.ascontiguousarray(np.asarray(Wp, np.float32).T.astype(bf))
    bp2 = np.ascontiguousarray(np.asarray(bp, np.float32).reshape(1, C).astype(bf))
    in_maps = []
    for b in range(B):
        in_maps.append(
            {
                "xt": np.ascontiguousarray(x[b].T.astype(bf)),
                "wq": wqT,
                "wk": wkT,
                "wv": wvT,
                "wp": wpT,
                "bp": bp2,
            }
        )
    return in_maps


def run(x, Wq, Wk, Wv, Wp, bp, trace=False):
    from concourse.bass_utils import run_bass_kernel_spmd

    nc = get_program()
    in_maps = make_in_maps(x, Wq, Wk, Wv, Wp, bp)
    res = run_bass_kernel_spmd(nc, in_maps, list(range(B)), trace=trace)
    out = np.stack([res.results[b]["out"] for b in range(B)])
    attn = np.stack([res.results[b]["attn"] for b in range(B)])
    return (out, attn), res


def kernel(x, Wq, Wk, Wv, Wp, bp):
    (out, attn), _ = run(x, Wq, Wk, Wv, Wp, bp, trace=False)
    return (out, attn)


# revision 29
# speedup vs baseline: 1.1123x; 1.1123x over previous
"""Multi-head attention Trainium2 kernel (B=8, N=1024, C=768, H=12, D=64).

Sharding: data-parallel over batch — one batch element per NeuronCore (8 cores).
Returns (out [8,1024,768], attn [8,12,1024,1024]) matching the reference.

Layout strategy (per core, batch b):
  - host passes x[b].T and W.T (bf16) so every matmul contracts over partitions
  - qT,kT [C,N] and v [N,C] via projections; per head pair the S and S^T
    logits are computed packed two-heads-per-PE-array (row groups 0-1 / 2-3)
  - attn = exp(S*scale) * recip(rowsum) with rowsums from ACT accum_out
  - attn@v uses exp(S^T) tiles; an all-ones lhsT col-packed matmul makes the
    PE broadcast the softmax sums into the same psum partitions for a
    partition-aligned normalize
  - out = attn_out @ Wp.T + bias via K=1 ones-row matmul
"""

import numpy as np

B, N, C, H, D = 8, 1024, 768, 12, 64
SCALE = D ** -0.5  # 0.125
NT = N // 128  # 8 n-tiles
CT = C // 128  # 6 c-tiles (= head pairs)

_CACHE = {}


def _build_program(bench_loop=None, skip_attn_store=False, skip_s_exp=False, no_accum=False):
    from contextlib import ExitStack

    import concourse.bacc as bacc
    import concourse.mybir as mybir
    import concourse.tile as tile

    fp32 = mybir.dt.float32
    bf16 = mybir.dt.bfloat16
    Exp = mybir.ActivationFunctionType.Exp

    nc = bacc.Bacc()
    xt = nc.declare_dram_parameter("xt", [C, N], bf16, isOutput=False)  # x[b].T
    wq = nc.declare_dram_parameter("wq", [C, C], bf16, isOutput=False)  # Wq.T
    wk = nc.declare_dram_parameter("wk", [C, C], bf16, isOutput=False)  # Wk.T
    wv = nc.declare_dram_parameter("wv", [C, C], bf16, isOutput=False)  # Wv.T
    wp = nc.declare_dram_parameter("wp", [C, C], bf16, isOutput=False)  # Wp.T
    bp = nc.declare_dram_parameter("bp", [1, C], bf16, isOutput=False)
    out = nc.declare_dram_parameter("out", [N, C], fp32, isOutput=True)
    attn = nc.declare_dram_parameter("attn", [H, N, N], fp32, isOutput=True)

    with ExitStack() as ctx:
        tc = ctx.enter_context(tile.TileContext(nc))
        if bench_loop is not None:
            ctx.enter_context(
                tc.For_i(0, bench_loop, 1,
                         hint_engines=(mybir.EngineType.PE,
                                       mybir.EngineType.Activation,
                                       mybir.EngineType.DVE))
            )

        qk_pool = ctx.enter_context(tc.tile_pool(name="qk", bufs=6))
        v_pool = ctx.enter_context(tc.tile_pool(name="vp", bufs=8))
        aot_pool = ctx.enter_context(tc.tile_pool(name="aot", bufs=6))
        wpb_pool = ctx.enter_context(tc.tile_pool(name="wpb", bufs=6))
        const_pool = ctx.enter_context(tc.tile_pool(name="cst", bufs=1))
        xw_pool = ctx.enter_context(tc.tile_pool(name="xw", bufs=6))
        ph2 = ctx.enter_context(tc.tile_pool(name="ph2", bufs=1))
        sml = ctx.enter_context(tc.tile_pool(name="sml", bufs=8))
        rrp = ctx.enter_context(tc.tile_pool(name="rrp", bufs=4))

        # PSUM: "s" 3x[128,1024] = 6 banks; "av" 2x[128,512] = 2 banks
        ps_pool = ctx.enter_context(tc.tile_pool(name="ps", bufs=3, space="PSUM"))
        av_pool = ctx.enter_context(tc.tile_pool(name="avp", bufs=2, space="PSUM"))

        qT = [qk_pool.tile([128, N], bf16, tag="qT", name=f"qT{i}") for i in range(CT)]
        kT = [qk_pool.tile([128, N], bf16, tag="kT", name=f"kT{i}") for i in range(CT)]
        v_sb = [v_pool.tile([128, C], bf16, tag="v", name=f"v{i}") for i in range(NT)]
        aot = [aot_pool.tile([128, N], bf16, tag="aot", name=f"aot{i}") for i in range(CT)]
        wp_sb = [wpb_pool.tile([128, C], bf16, tag="wp", name=f"wp{i}") for i in range(CT)]
        bp_sb = wpb_pool.tile([1, C], bf16, tag="bp", bufs=1)
        ones64 = const_pool.tile([128, 64], bf16, tag="o64", bufs=1)
        onesrow = const_pool.tile([1, N], bf16, tag="orow", bufs=1)
        xt_sb = [xw_pool.tile([128, N], bf16, tag="xt", name=f"xts{i}") for i in range(CT)]
        wq_sb = [xw_pool.tile([128, C], bf16, tag="wq", name=f"wqs{i}") for i in range(CT)]
        wk_sb = [xw_pool.tile([128, C], bf16, tag="wk", name=f"wks{i}") for i in range(CT)]
        wv_sb = [xw_pool.tile([128, C], bf16, tag="wv", name=f"wvs{i}") for i in range(CT)]

        nc.vector.memset(ones64, 1.0)
        nc.vector.memset(onesrow, 1.0)
        for t in range(CT):
            nc.sync.dma_start(out=xt_sb[t], in_=xt[t * 128 : (t + 1) * 128, :])
            nc.sync.dma_start(out=wq_sb[t], in_=wq[t * 128 : (t + 1) * 128, :])
            nc.sync.dma_start(out=wk_sb[t], in_=wk[t * 128 : (t + 1) * 128, :])
        nc.sync.dma_start(out=bp_sb, in_=bp[:, :])
        for t in range(CT):
            nc.sync.dma_start(out=wv_sb[t], in_=wv[t * 128 : (t + 1) * 128, :])
            nc.sync.dma_start(out=wp_sb[t], in_=wp[t * 128 : (t + 1) * 128, :])

        def qk_group(ct, g):
            # g in 0..3: (c2, q-or-k) = (g//2, g%2)
            csl = slice(ct * 128, (ct + 1) * 128)
            c2, which = g // 2, g % 2
            nsl = slice(c2 * 512, (c2 + 1) * 512)
            w_sb, dst = (wq_sb, qT) if which == 0 else (wk_sb, kT)
            p = ps_pool.tile([128, N], fp32, tag="s", name=f"p{which}{ct}{c2}")
            for kt_ in range(CT):
                nc.tensor.matmul(
                    p[:, 0:512], w_sb[kt_][:, csl], xt_sb[kt_][:, nsl],
                    start=(kt_ == 0), stop=(kt_ == CT - 1),
                )
            nc.vector.tensor_copy(dst[ct][:, nsl], p[:, 0:512])

        def qk_proj(ct):
            for g in range(4):
                qk_group(ct, g)

        def v_proj_mt(mt):
            pv = ps_pool.tile([128, C], fp32, tag="s", name=f"pv{mt}")
            msl = slice(mt * 128, (mt + 1) * 128)
            for kt_ in range(CT):
                nc.tensor.matmul(
                    pv[:, 0:512], xt_sb[kt_][:, msl], wv_sb[kt_][:, 0:512],
                    start=(kt_ == 0), stop=(kt_ == CT - 1),
                )
                nc.tensor.matmul(
                    pv[:, 512:768], xt_sb[kt_][:, msl], wv_sb[kt_][:, 512:768],
                    start=(kt_ == 0), stop=(kt_ == CT - 1),
                )
            nc.vector.tensor_copy(v_sb[mt], pv)

        def s_side(hp, extra=None, every=2):
            qt_t, kt_t = qT[hp], kT[hp]
            for nt in range(NT):
                if extra is not None and nt % every == every - 1:
                    extra(nt // every)
                nsl = slice(nt * 128, (nt + 1) * 128)
                pse = ps_pool.tile([128, N], fp32, tag="s", name=f"se{hp}{nt}")
                pso = ps_pool.tile([128, N], fp32, tag="s", name=f"so{hp}{nt}")
                for c2 in range(2):
                    sl = slice(c2 * 512, (c2 + 1) * 512)
                    nc.tensor.matmul(
                        pse[:, sl], qt_t[0:64, nsl], kt_t[0:64, sl],
                        start=True, stop=True, tile_position=(0, 0),
                    )
                    nc.tensor.matmul(
                        pso[:, sl], qt_t[64:128, nsl], kt_t[64:128, sl],
                        start=True, stop=True, tile_position=(64, 0),
                    )
                for h2, pst in ((0, pse), (1, pso)):
                    if skip_s_exp:
                        continue
                    h = 2 * hp + h2
                    ex = ph2.tile([128, N], fp32, tag="ex", bufs=6, name=f"ex{h}{nt}")
                    sm = sml.tile([128, 1], fp32, tag="sm", name=f"sm{h}{nt}")
                    if no_accum:
                        nc.scalar.activation(out=ex, in_=pst, func=Exp, scale=SCALE)
                        nc.vector.memset(sm, 1.0)
                    else:
                        nc.scalar.activation(
                            out=ex, in_=pst, func=Exp, scale=SCALE, accum_out=sm
                        )
                    rs = sml.tile([128, 1], fp32, tag="rs", name=f"rs{h}{nt}")
                    nc.vector.reciprocal_approx_fast(rs, sm)
                    nc.vector.tensor_scalar_mul(ex, ex, rs)
                    if not skip_attn_store:
                        nc.sync.dma_start(out=attn[h, nsl, :], in_=ex)

        def st_mms(hp, mt, ests):
            qt_t, kt_t = qT[hp], kT[hp]
            msl = slice(mt * 128, (mt + 1) * 128)
            pse = ps_pool.tile([128, N], fp32, tag="s", name=f"te{hp}{mt}")
            pso = ps_pool.tile([128, N], fp32, tag="s", name=f"to{hp}{mt}")
            for c2 in range(2):
                sl = slice(c2 * 512, (c2 + 1) * 512)
                nc.tensor.matmul(
                    pse[:, sl], kt_t[0:64, msl], qt_t[0:64, sl],
                    start=True, stop=True, tile_position=(0, 0),
                )
                nc.tensor.matmul(
                    pso[:, sl], kt_t[64:128, msl], qt_t[64:128, sl],
                    start=True, stop=True, tile_position=(64, 0),
                )
            este = ph2.tile([128, N], bf16, tag="est", bufs=22, name=f"ee{hp}{mt}")
            esto = ph2.tile([128, N], bf16, tag="est", bufs=22, name=f"eo{hp}{mt}")
            nc.scalar.activation(out=este, in_=pse, func=Exp, scale=SCALE)
            nc.scalar.activation(out=esto, in_=pso, func=Exp, scale=SCALE)
            ests.append((este, esto))

        def av_quarter(hp, ests, q, state):
            # q in 0..3: (chunk, mt-half) = (q//2, q%2)
            c2, half = q // 2, q % 2
            sl = slice(c2 * 512, (c2 + 1) * 512)
            if half == 0:
                avd = av_pool.tile([128, 512], fp32, tag="av", name=f"ad{hp}{c2}")
                avs = av_pool.tile([128, 512], fp32, tag="av", name=f"as{hp}{c2}")
                state[c2] = (avd, avs)
            avd, avs = state[c2]
            for mt in range(half * 4, half * 4 + 4):
                for h2 in range(2):
                    est_t = ests[mt][h2]
                    h = 2 * hp + h2
                    rsl = slice(h2 * 64, (h2 + 1) * 64)
                    nc.tensor.matmul(
                        avd[rsl, :],
                        v_sb[mt][:, h * 64 : (h + 1) * 64],
                        est_t[:, sl],
                        start=(mt == 0), stop=(mt == NT - 1),
                        tile_position=(0, h2 * 64),
                    )
                    nc.tensor.matmul(
                        avs[rsl, :], ones64, est_t[:, sl],
                        start=(mt == 0), stop=(mt == NT - 1),
                        tile_position=(0, h2 * 64),
                    )
            if half == 1:
                rr = rrp.tile([128, 512], fp32, tag="rr", name=f"rr{hp}{c2}")
                nc.vector.reciprocal_approx_fast(rr, avs)
                nc.vector.tensor_mul(aot[hp][:, sl], avd, rr)

        partials = []

        def proj_partial(nt):
            # pairs 0..4 contribution of the output projection (runs while
            # pair 5 is still computing); aot[5] + bias join in the tail
            nsl = slice(nt * 128, (nt + 1) * 128)
            po = ps_pool.tile([128, C], fp32, tag="s", name=f"pp{nt}")
            for c2, sl in ((0, slice(0, 512)), (1, slice(512, 768))):
                for kt_ in range(CT - 1):
                    nc.tensor.matmul(
                        po[:, sl], aot[kt_][:, nsl], wp_sb[kt_][:, sl],
                        start=(kt_ == 0), stop=(kt_ == CT - 2),
                    )
            pt = ph2.tile([128, C], bf16, tag="pt", bufs=8, name=f"pt{nt}")
            nc.vector.tensor_copy(pt, po)
            partials.append(pt)

        # software-pipelined schedule:
        #   pair t's S^T loop hosts AV of pair t-1 (V projection for pair 0);
        #   pair t's S loop hosts the QK projection of pair t+1 (or, for the
        #   last pair, the first 5/6 of the output projection).
        qk_proj(0)
        prev = None  # (hp, ests) awaiting AV
        for hp in range(CT):
            ests = []
            avstate = {}
            for mt in range(NT):
                st_mms(hp, mt, ests)
                if hp == 0:
                    v_proj_mt(mt)
                elif mt < 4:
                    av_quarter(prev[0], prev[1], mt, avstate)
            if hp + 1 < CT:
                s_loop_extra = lambda g: qk_group(hp + 1, g)
            else:
                s_loop_extra = lambda g: (proj_partial(2 * g), proj_partial(2 * g + 1))
            s_side(hp, s_loop_extra)
            prev = (hp, ests)
        avstate = {}
        for q in range(4):
            av_quarter(prev[0], prev[1], q, avstate)

        # output projection tail: last pair's contribution + bias + partial
        for nt in range(NT):
            nsl = slice(nt * 128, (nt + 1) * 128)
            po = ps_pool.tile([128, C], fp32, tag="s", name=f"po{nt}")
            for c2, sl in ((0, slice(0, 512)), (1, slice(512, 768))):
                nc.tensor.matmul(
                    po[:, sl], aot[CT - 1][:, nsl], wp_sb[CT - 1][:, sl],
                    start=True, stop=False,
                )
                nc.tensor.matmul(
                    po[:, sl], onesrow[0:1, nsl], bp_sb[0:1, sl],
                    start=False, stop=True,
                )
            ot = ph2.tile([128, C], fp32, tag="ot", bufs=3, name=f"ot{nt}")
            nc.vector.tensor_add(ot, po, partials[nt])
            nc.sync.dma_start(out=out[nsl, :], in_=ot)

    nc.compile()
    return nc


def get_program(bench_loop=None, **kw):
    key = ("nc", bench_loop, tuple(sorted(kw.items())))
    if key not in _CACHE:
        _CACHE[key] = _build_program(bench_loop, **kw)
    return _CACHE[key]


def make_in_maps(x, Wq, Wk, Wv, Wp, bp):
    import ml_dtypes

    bf = ml_dtypes.bfloat16
    x = np.asarray(x, dtype=np.float32)
    wqT = np.ascontiguousarray(np.asarray(Wq, np.float32).T.astype(bf))
    wkT = np.ascontiguousarray(np.asarray(Wk, np.float32).T.astype(bf))
    wvT = np.ascontiguousarray(np.asarray(Wv, np.float32).T.astype(bf))
    wpT = np.ascontiguousarray(np.asarray(Wp, np.float32).T.astype(bf))
    bp2 = np.ascontiguousarray(np.asarray(bp, np.float32).reshape(1, C).astype(bf))
    in_maps = []
    for b in range(B):
        in_maps.append(
            {
                "xt": np.ascontiguousarray(x[b].T.astype(bf)),
                "wq": wqT,
                "wk": wkT,
                "wv": wvT,
                "wp": wpT,
                "bp": bp2,
            }
        )
    return in_maps


def run(x, Wq, Wk, Wv, Wp, bp, trace=False):
    from concourse.bass_utils import run_bass_kernel_spmd

    nc = get_program()
    in_maps = make_in_maps(x, Wq, Wk, Wv, Wp, bp)
    res = run_bass_kernel_spmd(nc, in_maps, list(range(B)), trace=trace)
    out = np.stack([res.results[b]["out"] for b in range(B)])
    attn = np.stack([res.results[b]["attn"] for b in range(B)])
    return (out, attn), res


def _jax_has_neuron():
    try:
        import jax

        return any("NC_" in str(d) or d.platform == "axon" for d in jax.devices())
    except Exception:
        return False


def _run_subprocess(x, Wq, Wk, Wv, Wp, bp):
    """Fallback when the calling process pinned jax to CPU (e.g.
    JAX_PLATFORMS=cpu): rerun in a clean-env subprocess that can see the
    NeuronCores."""
    import os
    import subprocess
    import sys
    import tempfile

    with tempfile.TemporaryDirectory() as td:
        inp = os.path.join(td, "in.npz")
        outp = os.path.join(td, "out.npz")
        np.savez(inp, x=x, Wq=Wq, Wk=Wk, Wv=Wv, Wp=Wp, bp=bp)
        env = {k: v for k, v in os.environ.items() if k != "JAX_PLATFORMS"}
        script = (
            "import numpy as np, importlib.util\n"
            f"spec = importlib.util.spec_from_file_location('knl', {__file__!r})\n"
            "m = importlib.util.module_from_spec(spec); spec.loader.exec_module(m)\n"
            f"d = np.load({inp!r})\n"
            "out, attn = m.kernel(**{k: d[k] for k in d.files})\n"
            f"np.savez({outp!r}, out=out, attn=attn)\n"
        )
        subprocess.run([sys.executable, "-c", script], check=True, env=env)
        d = np.load(outp)
        return d["out"], d["attn"]


def kernel(x, Wq, Wk, Wv, Wp, bp):
    if not _jax_has_neuron():
        out, attn = _run_subprocess(
            np.asarray(x, np.float32), np.asarray(Wq, np.float32),
            np.asarray(Wk, np.float32), np.asarray(Wv, np.float32),
            np.asarray(Wp, np.float32), np.asarray(bp, np.float32),
        )
        return (out, attn)
    (out, attn), _ = run(x, Wq, Wk, Wv, Wp, bp, trace=False)
    return (out, attn)
